# revision 4
# baseline (speedup 1.0000x reference)
"""DKVMN forward kernel v2 — bf16 big arrays + PSUM-accumulated reads.

Layout per (student b, m-half h): chains of CH=202 slots per m
([states t=0..199 | pad | reset-slot]); the scan output S is left-padded by
one element so the shifted read S[m*CH + t] = Mv_{t-1}[m] works for every
(m, t) including t=0 (chain m reads chain m-1's reset slot, chain 0 reads the
pre-written pad cell). The m-reduction of reads happens inside the
f = tanh(fW1 r + fW2 k) matmul: 50 per-m matmuls accumulate in PSUM.
"""
import os
import sys

sys.path.insert(0, "/opt/trn_rl_repo")

import numpy as np
import ml_dtypes

import concourse.bass as bass
import concourse.mybir as mybir
from concourse import bass_utils, tile
from concourse.bacc import Bacc

B, T, NUM_C, D, M = 64, 200, 2000, 128, 50
N_CORES = 8
BC = B // N_CORES
BT = BC * T                  # 1600
NCHUNK = (BT + 127) // 128   # 13
BTP = NCHUNK * 128           # 1664
MH = M // 2                  # 25
CH = T + 2                   # 202: states + pad + reset slot
FP32 = mybir.dt.float32
BF16 = mybir.dt.bfloat16
INT32 = mybir.dt.int32

_COMPILED = {}


def _build_nc():
    nc = Bacc("TRN2", target_bir_lowering=False, debug=False, num_devices=N_CORES)

    din = {}
    def dram_in(name, shape, dtype=FP32):
        din[name] = nc.dram_tensor(name, shape, dtype, kind="ExternalInput")
        return din[name]

    dram_in("kidx", [128, NCHUNK], INT32)
    dram_in("vidx", [128, NCHUNK], INT32)
    dram_in("pidx", [128, NCHUNK], INT32)
    dram_in("k_emb", [NUM_C + 1, D])
    dram_in("v_emb", [2 * NUM_C + 1, D])
    dram_in("p_W", [NUM_C, D])
    dram_in("MkT", [D, M])
    dram_in("Mv0T", [D, M])
    dram_in("eWT", [D, D])
    dram_in("aWT", [D, D])
    dram_in("fW1Tb", [D, D], BF16)
    dram_in("fW2Tb", [D, D], BF16)
    dram_in("e_b", [D, 1])
    dram_in("a_b", [D, 1])
    dram_in("f_b", [D, 1])
    dram_in("ident", [D, D])
    dram_in("identb", [D, D], BF16)
    dram_in("ones", [D, 1])
    dram_in("ones_row", [1, D], BF16)
    dram_in("pb_sel", [1, BT])
    out_d = nc.dram_tensor("out", [BC, T], FP32, kind="ExternalOutput")

    AL = mybir.AluOpType
    AF = mybir.ActivationFunctionType

    with tile.TileContext(nc) as tc:
        with (
            tc.tile_pool(name="const", bufs=1) as cpool,
            tc.tile_pool(name="ph1", bufs=1) as ph1,
            tc.tile_pool(name="rows", bufs=3) as rows_p,
            tc.tile_pool(name="wtile", bufs=1) as wt_p,
            tc.tile_pool(name="blk", bufs=2) as blk,
            tc.tile_pool(name="wmp", bufs=2) as wmp_p,
            tc.tile_pool(name="wbc", bufs=2) as wbc_p,
            tc.tile_pool(name="small", bufs=4) as sm,
            tc.tile_pool(name="psum", bufs=2, space="PSUM") as pp,
            tc.tile_pool(name="psumT", bufs=2, space="PSUM") as ppT,
            tc.tile_pool(name="psumF", bufs=2, space="PSUM") as ppF,
        ):
            def load_const(name, shape, dtype=FP32):
                # small consts dispatch from the scalar queue so the sync
                # queue carries only the big streams
                t = cpool.tile(shape, dtype, tag=name, name=name + "_sb")
                nc.scalar.dma_start(t[:], din[name].ap())
                return t

            kidx = load_const("kidx", [128, NCHUNK], INT32)
            vidx = load_const("vidx", [128, NCHUNK], INT32)
            pidx = load_const("pidx", [128, NCHUNK], INT32)
            MkT = load_const("MkT", [D, M])
            Mv0T = load_const("Mv0T", [D, M])
            eWT = load_const("eWT", [D, D])
            aWT = load_const("aWT", [D, D])
            fW1Tb = load_const("fW1Tb", [D, D], BF16)
            fW2Tb = load_const("fW2Tb", [D, D], BF16)
            e_b = load_const("e_b", [D, 1])
            a_b = load_const("a_b", [D, 1])
            f_b = load_const("f_b", [D, 1])
            ident = load_const("ident", [D, D])
            identb = load_const("identb", [D, D], BF16)
            ones = load_const("ones", [D, 1])
            ones_row = load_const("ones_row", [1, D], BF16)
            pb_sel = load_const("pb_sel", [1, BT])

            # ---- phase-1 tiles (filled per chunk, interleaved with pairs) ----
            k_T = ph1.tile([D, BTP], FP32, tag="k_T")
            v_T = ph1.tile([D, BTP], FP32, tag="v_T")
            pw_T = ph1.tile([D, BTP], FP32, tag="pw_T")
            k_Tb = ph1.tile([D, BT], BF16, tag="k_Tb")
            w_Tm = ph1.tile([M, BTP], BF16, tag="w_Tm")
            # +2 pad cols: per-student CH-wide windows may read 2 cols past
            # the last student's range (multiplied by w=0, never used)
            e_sb = ph1.tile([D, BT + 2], BF16, tag="e_sb")
            a_sb = ph1.tile([D, BT + 2], BF16, tag="a_sb")

            def phase1_chunk(j):
                c0 = j * 128
                for table, idxt, dst in (
                    ("k_emb", kidx, k_T),
                    ("v_emb", vidx, v_T),
                    ("p_W", pidx, pw_T),
                ):
                    r = rows_p.tile([128, D], FP32, tag="rows", name=f"r{j}")
                    nc.gpsimd.indirect_dma_start(
                        out=r[:],
                        out_offset=None,
                        in_=din[table].ap(),
                        in_offset=bass.IndirectOffsetOnAxis(ap=idxt[:, j : j + 1], axis=0),
                    )
                    pt = ppT.tile([128, D], FP32, tag="tp", name=f"pt{j}")
                    nc.tensor.transpose(out=pt[:], in_=r[:], identity=ident[:])
                    nc.scalar.copy(dst[:, c0 : c0 + 128], pt[:])
                # softmax over slots for this chunk
                pw = pp.tile([128, M], FP32, tag="mm", name=f"pw{j}")
                nc.tensor.matmul(pw[:], lhsT=k_T[:, c0 : c0 + 128], rhs=MkT[:])
                nmax = sm.tile([128, 1], FP32, tag="nmax", name=f"nmax{j}")
                nc.vector.tensor_reduce(nmax[:], pw[:], axis=mybir.AxisListType.X,
                                        op=AL.max, negate=True)
                wt = wt_p.tile([128, M], BF16, tag=f"w{j}", name=f"wt{j}")
                sume = sm.tile([128, 1], FP32, tag="sume", name=f"sume{j}")
                nc.scalar.activation(wt[:], pw[:], AF.Exp, bias=nmax[:], scale=1.0,
                                     accum_out=sume[:])
                rinv = sm.tile([128, 1], FP32, tag="rinv", name=f"rinv{j}")
                nc.vector.reciprocal(rinv[:], sume[:])
                nc.vector.tensor_scalar_mul(wt[:], wt[:], rinv[:])
                ptw = ppT.tile([M, 128], BF16, tag="tp", name=f"ptw{j}")
                nc.tensor.transpose(out=ptw[:], in_=wt[:], identity=identb[:])
                nc.scalar.copy(w_Tm[:, c0 : c0 + 128], ptw[:])

            def pw_chunk(j):
                c0 = j * 128
                r = rows_p.tile([128, D], FP32, tag="rows", name=f"rp{j}")
                nc.gpsimd.indirect_dma_start(
                    out=r[:],
                    out_offset=None,
                    in_=din["p_W"].ap(),
                    in_offset=bass.IndirectOffsetOnAxis(ap=pidx[:, j : j + 1], axis=0),
                )
                pt = ppT.tile([128, D], FP32, tag="tp", name=f"ptp{j}")
                nc.tensor.transpose(out=pt[:], in_=r[:], identity=ident[:])
                nc.scalar.copy(pw_T[:, c0 : c0 + 128], pt[:])

            def phase1_tail(p):
                # e/a projections + k cast for pair p's 400 cols (+2 overlap)
                c = 400 * p
                cw = min(402, BTP - c)
                for (wmat, bias, func, dst) in (
                    (eWT, e_b, AF.Sigmoid, e_sb),
                    (aWT, a_b, AF.Tanh, a_sb),
                ):
                    pe_ = pp.tile([D, 402], FP32, tag="mm", name=f"pe{p}{func}")
                    nc.tensor.matmul(pe_[:, 0:cw], lhsT=wmat[:], rhs=v_T[:, c : c + cw])
                    nc.scalar.activation(dst[:, c : c + cw], pe_[:, 0:cw], func,
                                         bias=bias[:], scale=1.0)
                nc.vector.tensor_copy(k_Tb[:, c : c + 400], k_T[:, c : c + 400])

            # ---- scan blocks (full CH=202 grid; all big ops contiguous) ----
            # Students processed in pairs: per h, both students' wm land in a
            # shared [128, MH, 2*CH] tile so each read-matmul covers N=404.
            LG = MH * CH  # 5050
            PG = 2 * CH   # 404
            f_sb = ph1.tile([D, BT], FP32, tag="f_sb")

            def chains(b, h, wm_pair, slot):
                m0 = h * MH
                # w on the full grid: pads zeroed so downstream products
                # vanish there
                w_flat = wbc_p.tile([1, LG], BF16, tag="w_flat", name="w_flat")
                wf3 = w_flat[:].rearrange("p (m t) -> p m t", m=MH)
                nc.vector.memset(wf3[:, :, T:CH], 0.0)
                nc.sync.dma_start(
                    wf3[:, :, 0:T],
                    w_Tm[m0 : m0 + MH, b * T : (b + 1) * T],
                )
                w_bc = wbc_p.tile([128, LG], BF16, tag="w_bc", name="w_bc")
                for c0 in range(0, LG, 1024):
                    cw = min(1024, LG - c0)
                    pb_ps = pp.tile([128, 1024], FP32, tag="mm", name="pb_ps")
                    nc.tensor.matmul(pb_ps[:, 0:512], lhsT=ones_row[:],
                                     rhs=w_flat[:, c0 : c0 + 512])
                    nc.tensor.matmul(pb_ps[:, 512:cw], lhsT=ones_row[:],
                                     rhs=w_flat[:, c0 + 512 : c0 + cw])
                    nc.scalar.copy(w_bc[:, c0 : c0 + cw], pb_ps[:, 0:cw])
                w_bc3 = w_bc[:].rearrange("p (m t) -> p m t", m=MH)

                alpha = blk.tile([128, LG], BF16, tag="alpha", name="alpha")
                beta = blk.tile([128, LG], BF16, tag="beta", name="beta")
                al3 = alpha[:].rearrange("p (m t) -> p m t", m=MH)
                be3 = beta[:].rearrange("p (m t) -> p m t", m=MH)
                e_bc = e_sb[:, b * T : b * T + CH].unsqueeze(1).broadcast_to([D, MH, CH])
                a_bc = a_sb[:, b * T : b * T + CH].unsqueeze(1).broadcast_to([D, MH, CH])
                # u = w*e on the full grid (pads -> 0 since w=0 there)
                nc.vector.tensor_tensor(al3, w_bc3, e_bc, op=AL.mult)
                # alpha = 1-u on scalar engine, then re-zero pad+reset cols
                nc.scalar.activation(alpha[:], alpha[:], AF.Copy, bias=1.0,
                                     scale=-1.0)
                nc.vector.memset(al3[:, :, T : T + 2], 0.0)
                # beta = w*a (pads 0), reset slot = next chain's Mv0
                nc.vector.tensor_tensor(be3, w_bc3, a_bc, op=AL.mult)
                if MH > 1:
                    nc.scalar.copy(
                        be3[:, 0 : MH - 1, T + 1 : T + 2],
                        Mv0T[:, m0 + 1 : m0 + MH].rearrange("p (m o) -> p m o", o=1),
                    )

                S = blk.tile([128, 1 + LG], BF16, tag="S", name="S")
                nc.scalar.copy(S[:, 0:1], Mv0T[:, m0 : m0 + 1])
                nc.vector.tensor_tensor_scan(
                    S[:, 1 : 1 + LG], alpha[:], beta[:],
                    Mv0T[:, m0 : m0 + 1], op0=AL.mult, op1=AL.add
                )
                # wm = S_shift * w: contiguous ins, strided slot in wm_pair
                wm3 = wm_pair[:].rearrange("p (m t) -> p m t", m=MH)[
                    :, :, slot * CH : slot * CH + CH]
                nc.vector.tensor_tensor(
                    wm3, S[:, 0:LG].rearrange("p (m t) -> p m t", m=MH),
                    w_bc3, op=AL.mult)

            pred = ph1.tile([1, BT], FP32, tag="pred")
            pair_state = {}

            def pair_h(p, h):
                b0 = 2 * p
                if h == 0:
                    pair_state[p] = (
                        ppF.tile([D, PG], FP32, tag="fps", name=f"f_ps{p}"),
                        wmp_p.tile([128, MH * PG], BF16, tag="wmp", name=f"wmp{p}"),
                    )
                f_ps, wm_pair = pair_state[p]
                wmp3 = wm_pair[:].rearrange("p (m t) -> p m t", m=MH)
                chains(b0, h, wm_pair, 0)
                chains(b0 + 1, h, wm_pair, 1)
                for m in range(MH):
                    nc.tensor.matmul(
                        f_ps[:], lhsT=fW1Tb[:], rhs=wmp3[:, m, :],
                        start=(h == 0 and m == 0), stop=False)
                if h == 1:
                    # + fW2 @ k per student, close the accumulation
                    nc.tensor.matmul(f_ps[:, 0:T], lhsT=fW2Tb[:],
                                     rhs=k_Tb[:, b0 * T : (b0 + 1) * T],
                                     start=False, stop=False)
                    nc.tensor.matmul(f_ps[:, CH : CH + T], lhsT=fW2Tb[:],
                                     rhs=k_Tb[:, (b0 + 1) * T : (b0 + 2) * T],
                                     start=False, stop=True)
                    nc.scalar.activation(f_sb[:, b0 * T : (b0 + 1) * T],
                                         f_ps[:, 0:T],
                                         AF.Tanh, bias=f_b[:], scale=1.0)
                    nc.scalar.activation(f_sb[:, (b0 + 1) * T : (b0 + 2) * T],
                                         f_ps[:, CH : CH + T],
                                         AF.Tanh, bias=f_b[:], scale=1.0)


            # interleaved schedule: chunk/tail units slotted between pair
            # halves so tensor-side phase-1 fills stalls inside each pair
            schedule = [
                ("c", 0), ("c", 1), ("c", 2), ("c", 3), ("t", 0),
                ("P", (0, 0)), ("c", 4), ("c", 5), ("P", (0, 1)),
                ("c", 6), ("t", 1),
                ("P", (1, 0)), ("c", 7), ("c", 8), ("P", (1, 1)),
                ("c", 9), ("t", 2),
                ("P", (2, 0)), ("c", 10), ("c", 11), ("P", (2, 1)),
                ("c", 12), ("t", 3),
                ("P", (3, 0)), ("P", (3, 1)),
            ]
            for kind, arg in schedule:
                if kind == "c":
                    phase1_chunk(arg)
                elif kind == "t":
                    phase1_tail(arg)
                elif kind == "w":
                    pw_chunk(arg)
                else:
                    pair_h(*arg)

            # ---- pred = sigmoid(sum_d f*pw + pb) ----
            nc.vector.tensor_mul(f_sb[:], f_sb[:], pw_T[:, :BT])
            for c in range(0, BT, 400):
                ppd = pp.tile([128, 1024], FP32, tag="mm", name=f"ppd{c}")
                nc.tensor.matmul(ppd[0:1, 0:400], lhsT=ones[:],
                                 rhs=f_sb[:, c : c + 400])
                nc.scalar.copy(pred[:, c : c + 400], ppd[0:1, 0:400])
            nc.vector.tensor_add(pred[:], pred[:], pb_sel[:])
            nc.scalar.activation(pred[:], pred[:], AF.Sigmoid)
            nc.sync.dma_start(out_d.ap().rearrange("b t -> (b t)").unsqueeze(0), pred[:])

    nc.compile()
    return nc


def _prep_inputs(skill, answer, k_emb, v_emb, Mk, Mv0, f_W, f_b, p_W, p_b,
                 e_W, e_b, a_W, a_b):
    skill = np.asarray(skill)
    answer = np.asarray(answer)
    answer_x = np.where(answer == 2, 1, answer)
    x = (skill + NUM_C * answer_x).astype(np.int64)
    nxt = np.concatenate([skill[:, 1:], np.zeros((B, 1), skill.dtype)], axis=1)
    pidx_full = np.minimum(nxt, NUM_C - 1).astype(np.int64)

    def idx_tiles(a):
        flat = np.zeros(BTP, np.int32)
        flat[:BT] = a.reshape(-1).astype(np.int32)
        return np.ascontiguousarray(flat.reshape(NCHUNK, 128).T)

    bf = ml_dtypes.bfloat16
    common = {
        "k_emb": np.ascontiguousarray(k_emb, np.float32),
        "v_emb": np.ascontiguousarray(v_emb, np.float32),
        "p_W": np.ascontiguousarray(p_W, np.float32),
        "MkT": np.ascontiguousarray(Mk.T, np.float32),
        "Mv0T": np.ascontiguousarray(Mv0.T, np.float32),
        "eWT": np.ascontiguousarray(e_W.T, np.float32),
        "aWT": np.ascontiguousarray(a_W.T, np.float32),
        "fW1Tb": np.ascontiguousarray(f_W[:, :D].T).astype(bf),
        "fW2Tb": np.ascontiguousarray(f_W[:, D:].T).astype(bf),
        "e_b": np.ascontiguousarray(e_b.reshape(D, 1), np.float32),
        "a_b": np.ascontiguousarray(a_b.reshape(D, 1), np.float32),
        "f_b": np.ascontiguousarray(f_b.reshape(D, 1), np.float32),
        "ident": np.eye(D, dtype=np.float32),
        "identb": np.eye(D, dtype=np.float32).astype(bf),
        "ones": np.ones((D, 1), np.float32),
        "ones_row": np.ones((1, D)).astype(ml_dtypes.bfloat16),
    }
    in_maps = []
    for c in range(N_CORES):
        sl = slice(c * BC, (c + 1) * BC)
        m = dict(common)
        m["kidx"] = idx_tiles(skill[sl])
        m["vidx"] = idx_tiles(x[sl])
        m["pidx"] = idx_tiles(pidx_full[sl])
        m["pb_sel"] = np.ascontiguousarray(
            np.asarray(p_b, np.float32)[pidx_full[sl]].reshape(1, BT))
        in_maps.append(m)
    return in_maps


def _install_ntff_hook_shim():
    """Provide antenv.axon_hooks with a ctypes NTFF profile hook when the
    container's antenv package lacks it (needed only for trace=True)."""
    import types
    import ctypes
    import contextlib

    try:
        from antenv.axon_hooks import get_axon_ntff_profile_hook  # noqa: F401
        return
    except ImportError:
        pass

    so_path = "/opt/axon/libaxon_pjrt.so"
    hook = None
    if os.path.exists(so_path):
        lib = ctypes.CDLL(so_path)
        if hasattr(lib, "axon_start_nrt_profile"):
            lib.axon_start_nrt_profile.argtypes = [
                ctypes.POINTER(ctypes.c_int64), ctypes.c_size_t]
            lib.axon_start_nrt_profile.restype = ctypes.c_int64
            lib.axon_stop_nrt_profile.argtypes = [ctypes.c_char_p]
            lib.axon_stop_nrt_profile.restype = ctypes.c_int64

            @contextlib.contextmanager
            def _hook(output_dir, device_ids):
                import jax
                jax.devices()
                if device_ids:
                    ids = (ctypes.c_int64 * len(device_ids))(*device_ids)
                    rc = lib.axon_start_nrt_profile(ids, len(device_ids))
                else:
                    rc = lib.axon_start_nrt_profile(None, 0)
                if rc != 0:
                    raise RuntimeError(f"axon_start_nrt_profile rc={rc}")
                try:
                    yield
                finally:
                    n = lib.axon_stop_nrt_profile(str(output_dir).encode())
                    print(f"profile: {n} file(s) written to {output_dir}",
                          file=sys.stderr)

            hook = _hook

    mod = types.ModuleType("antenv.axon_hooks")
    mod._hook = hook
    mod.get_axon_ntff_profile_hook = lambda: mod._hook
    mod.set_axon_ntff_profile_hook = lambda h: setattr(mod, "_hook", h)
    import antenv
    antenv.axon_hooks = mod
    sys.modules["antenv.axon_hooks"] = mod




def kernel(**inputs):
    if "nc" not in _COMPILED:
        _COMPILED["nc"] = _build_nc()
    nc = _COMPILED["nc"]
    in_maps = _prep_inputs(**inputs)
    trace = bool(int(os.environ.get("KERNEL_TRACE", "0")))
    if trace:
        _install_ntff_hook_shim()
    res = bass_utils.run_bass_kernel_spmd(
        nc, in_maps, core_ids=list(range(N_CORES)), trace=trace
    )
    _COMPILED["last_result"] = res
    out = np.concatenate([res.results[c]["out"][:, : T - 1] for c in range(N_CORES)], axis=0)
    return out.astype(np.float32)



# revision 6
# speedup vs baseline: 1.0046x; 1.0046x over previous
"""DKVMN forward kernel v2 — bf16 big arrays + PSUM-accumulated reads.

Layout per (student b, m-half h): chains of CH=202 slots per m
([states t=0..199 | pad | reset-slot]); the scan output S is left-padded by
one element so the shifted read S[m*CH + t] = Mv_{t-1}[m] works for every
(m, t) including t=0 (chain m reads chain m-1's reset slot, chain 0 reads the
pre-written pad cell). The m-reduction of reads happens inside the
f = tanh(fW1 r + fW2 k) matmul: 50 per-m matmuls accumulate in PSUM.
"""
import os
import sys

sys.path.insert(0, "/opt/trn_rl_repo")

import numpy as np
import ml_dtypes

import concourse.bass as bass
import concourse.mybir as mybir
from concourse import bass_utils, tile
from concourse.bacc import Bacc

B, T, NUM_C, D, M = 64, 200, 2000, 128, 50
N_CORES = 8
BC = B // N_CORES
BT = BC * T                  # 1600
NCHUNK = (BT + 127) // 128   # 13
BTP = NCHUNK * 128           # 1664
MH = M // 2                  # 25
CH = T + 2                   # 202: states + pad + reset slot
FP32 = mybir.dt.float32
BF16 = mybir.dt.bfloat16
INT32 = mybir.dt.int32

_COMPILED = {}


def _build_nc():
    nc = Bacc("TRN2", target_bir_lowering=False, debug=False, num_devices=N_CORES)

    din = {}
    def dram_in(name, shape, dtype=FP32):
        din[name] = nc.dram_tensor(name, shape, dtype, kind="ExternalInput")
        return din[name]

    dram_in("kidx", [128, NCHUNK], INT32)
    dram_in("vidx", [128, NCHUNK], INT32)
    dram_in("pidx", [128, NCHUNK], INT32)
    dram_in("k_emb", [NUM_C + 1, D])
    dram_in("v_emb", [2 * NUM_C + 1, D])
    dram_in("p_W", [NUM_C, D])
    dram_in("MkT", [D, M])
    dram_in("Mv0T", [D, M])
    dram_in("eWT", [D, D])
    dram_in("aWT", [D, D])
    dram_in("fW1Tb", [D, D], BF16)
    dram_in("fW2Tb", [D, D], BF16)
    dram_in("e_b", [D, 1])
    dram_in("a_b", [D, 1])
    dram_in("f_b", [D, 1])
    dram_in("ident", [D, D])
    dram_in("identb", [D, D], BF16)
    dram_in("ones", [D, 1], BF16)
    dram_in("ones_row", [1, D], BF16)
    dram_in("pb_sel", [1, BT])
    out_d = nc.dram_tensor("out", [BC, T], FP32, kind="ExternalOutput")

    AL = mybir.AluOpType
    AF = mybir.ActivationFunctionType

    with tile.TileContext(nc) as tc:
        with (
            tc.tile_pool(name="const", bufs=1) as cpool,
            tc.tile_pool(name="ph1", bufs=1) as ph1,
            tc.tile_pool(name="rows", bufs=3) as rows_p,
            tc.tile_pool(name="wtile", bufs=1) as wt_p,
            tc.tile_pool(name="blk", bufs=2) as blk,
            tc.tile_pool(name="wmp", bufs=2) as wmp_p,
            tc.tile_pool(name="wbc", bufs=2) as wbc_p,
            tc.tile_pool(name="small", bufs=4) as sm,
            tc.tile_pool(name="psum", bufs=2, space="PSUM") as pp,
            tc.tile_pool(name="psumT", bufs=2, space="PSUM") as ppT,
            tc.tile_pool(name="psumF", bufs=2, space="PSUM") as ppF,
        ):
            def load_const(name, shape, dtype=FP32):
                # small consts dispatch from the scalar queue so the sync
                # queue carries only the big streams
                t = cpool.tile(shape, dtype, tag=name, name=name + "_sb")
                nc.scalar.dma_start(t[:], din[name].ap())
                return t

            kidx = load_const("kidx", [128, NCHUNK], INT32)
            vidx = load_const("vidx", [128, NCHUNK], INT32)
            pidx = load_const("pidx", [128, NCHUNK], INT32)
            MkT = load_const("MkT", [D, M])
            Mv0T = load_const("Mv0T", [D, M])
            eWT = load_const("eWT", [D, D])
            aWT = load_const("aWT", [D, D])
            fW1Tb = load_const("fW1Tb", [D, D], BF16)
            fW2Tb = load_const("fW2Tb", [D, D], BF16)
            e_b = load_const("e_b", [D, 1])
            a_b = load_const("a_b", [D, 1])
            f_b = load_const("f_b", [D, 1])
            ident = load_const("ident", [D, D])
            identb = load_const("identb", [D, D], BF16)
            ones = load_const("ones", [D, 1])
            ones_row = load_const("ones_row", [1, D], BF16)
            pb_sel = load_const("pb_sel", [1, BT])

            # ---- phase-1 tiles (filled per chunk, interleaved with pairs) ----
            k_T = ph1.tile([D, BTP], FP32, tag="k_T")
            v_T = ph1.tile([D, BTP], FP32, tag="v_T")
            pw_T = ph1.tile([D, BTP], FP32, tag="pw_T")
            k_Tb = ph1.tile([D, BT], BF16, tag="k_Tb")
            w_Tm = ph1.tile([M, BTP], BF16, tag="w_Tm")
            # +2 pad cols: per-student CH-wide windows may read 2 cols past
            # the last student's range (multiplied by w=0, never used)
            e_sb = ph1.tile([D, BT + 2], BF16, tag="e_sb")
            a_sb = ph1.tile([D, BT + 2], BF16, tag="a_sb")

            def phase1_chunk(j):
                c0 = j * 128
                for table, idxt, dst in (
                    ("k_emb", kidx, k_T),
                    ("v_emb", vidx, v_T),
                    ("p_W", pidx, pw_T),
                ):
                    r = rows_p.tile([128, D], FP32, tag="rows", name=f"r{j}")
                    nc.gpsimd.indirect_dma_start(
                        out=r[:],
                        out_offset=None,
                        in_=din[table].ap(),
                        in_offset=bass.IndirectOffsetOnAxis(ap=idxt[:, j : j + 1], axis=0),
                    )
                    pt = ppT.tile([128, D], FP32, tag="tp", name=f"pt{j}")
                    nc.tensor.transpose(out=pt[:], in_=r[:], identity=ident[:])
                    nc.scalar.copy(dst[:, c0 : c0 + 128], pt[:])
                # softmax over slots for this chunk
                pw = pp.tile([128, M], FP32, tag="mm", name=f"pw{j}")
                nc.tensor.matmul(pw[:], lhsT=k_T[:, c0 : c0 + 128], rhs=MkT[:])
                nmax = sm.tile([128, 1], FP32, tag="nmax", name=f"nmax{j}")
                nc.vector.tensor_reduce(nmax[:], pw[:], axis=mybir.AxisListType.X,
                                        op=AL.max, negate=True)
                wt = wt_p.tile([128, M], BF16, tag=f"w{j}", name=f"wt{j}")
                sume = sm.tile([128, 1], FP32, tag="sume", name=f"sume{j}")
                nc.scalar.activation(wt[:], pw[:], AF.Exp, bias=nmax[:], scale=1.0,
                                     accum_out=sume[:])
                rinv = sm.tile([128, 1], FP32, tag="rinv", name=f"rinv{j}")
                nc.vector.reciprocal(rinv[:], sume[:])
                nc.vector.tensor_scalar_mul(wt[:], wt[:], rinv[:])
                ptw = ppT.tile([M, 128], BF16, tag="tp", name=f"ptw{j}")
                nc.tensor.transpose(out=ptw[:], in_=wt[:], identity=identb[:])
                nc.scalar.copy(w_Tm[:, c0 : c0 + 128], ptw[:])

            def pw_chunk(j):
                c0 = j * 128
                r = rows_p.tile([128, D], FP32, tag="rows", name=f"rp{j}")
                nc.gpsimd.indirect_dma_start(
                    out=r[:],
                    out_offset=None,
                    in_=din["p_W"].ap(),
                    in_offset=bass.IndirectOffsetOnAxis(ap=pidx[:, j : j + 1], axis=0),
                )
                pt = ppT.tile([128, D], FP32, tag="tp", name=f"ptp{j}")
                nc.tensor.transpose(out=pt[:], in_=r[:], identity=ident[:])
                nc.scalar.copy(pw_T[:, c0 : c0 + 128], pt[:])

            def phase1_tail(p):
                # e/a projections + k cast for pair p's 400 cols (+2 overlap)
                c = 400 * p
                cw = min(402, BTP - c)
                for (wmat, bias, func, dst) in (
                    (eWT, e_b, AF.Sigmoid, e_sb),
                    (aWT, a_b, AF.Tanh, a_sb),
                ):
                    pe_ = pp.tile([D, 402], FP32, tag="mm", name=f"pe{p}{func}")
                    nc.tensor.matmul(pe_[:, 0:cw], lhsT=wmat[:], rhs=v_T[:, c : c + cw])
                    nc.scalar.activation(dst[:, c : c + cw], pe_[:, 0:cw], func,
                                         bias=bias[:], scale=1.0)
                nc.vector.tensor_copy(k_Tb[:, c : c + 400], k_T[:, c : c + 400])

            # ---- scan blocks (full CH=202 grid; all big ops contiguous) ----
            # Students processed in pairs: per h, both students' wm land in a
            # shared [128, MH, 2*CH] tile so each read-matmul covers N=404.
            LG = MH * CH  # 5050
            PG = 2 * CH   # 404
            f_sb = ph1.tile([D, BT], BF16, tag="f_sb")

            def chains(b, h, wm_pair, slot):
                m0 = h * MH
                # w on the full grid: pads zeroed so downstream products
                # vanish there
                w_flat = wbc_p.tile([1, LG], BF16, tag="w_flat", name="w_flat")
                wf3 = w_flat[:].rearrange("p (m t) -> p m t", m=MH)
                nc.vector.memset(wf3[:, :, T:CH], 0.0)
                nc.sync.dma_start(
                    wf3[:, :, 0:T],
                    w_Tm[m0 : m0 + MH, b * T : (b + 1) * T],
                )
                w_bc = wbc_p.tile([128, LG], BF16, tag="w_bc", name="w_bc")
                for c0 in range(0, LG, 1024):
                    cw = min(1024, LG - c0)
                    pb_ps = pp.tile([128, 1024], FP32, tag="mm", name="pb_ps")
                    nc.tensor.matmul(pb_ps[:, 0:512], lhsT=ones_row[:],
                                     rhs=w_flat[:, c0 : c0 + 512])
                    nc.tensor.matmul(pb_ps[:, 512:cw], lhsT=ones_row[:],
                                     rhs=w_flat[:, c0 + 512 : c0 + cw])
                    nc.scalar.copy(w_bc[:, c0 : c0 + cw], pb_ps[:, 0:cw])
                w_bc3 = w_bc[:].rearrange("p (m t) -> p m t", m=MH)

                alpha = blk.tile([128, LG], BF16, tag="alpha", name="alpha")
                beta = blk.tile([128, LG], BF16, tag="beta", name="beta")
                al3 = alpha[:].rearrange("p (m t) -> p m t", m=MH)
                be3 = beta[:].rearrange("p (m t) -> p m t", m=MH)
                e_bc = e_sb[:, b * T : b * T + CH].unsqueeze(1).broadcast_to([D, MH, CH])
                a_bc = a_sb[:, b * T : b * T + CH].unsqueeze(1).broadcast_to([D, MH, CH])
                # u = w*e on the full grid (pads -> 0 since w=0 there)
                nc.vector.tensor_tensor(al3, w_bc3, e_bc, op=AL.mult)
                # alpha = 1-u on scalar engine, then re-zero pad+reset cols
                nc.scalar.activation(alpha[:], alpha[:], AF.Copy, bias=1.0,
                                     scale=-1.0)
                nc.vector.memset(al3[:, :, T : T + 2], 0.0)
                # beta = w*a (pads 0), reset slot = next chain's Mv0
                nc.vector.tensor_tensor(be3, w_bc3, a_bc, op=AL.mult)
                if MH > 1:
                    nc.scalar.copy(
                        be3[:, 0 : MH - 1, T + 1 : T + 2],
                        Mv0T[:, m0 + 1 : m0 + MH].rearrange("p (m o) -> p m o", o=1),
                    )

                S = blk.tile([128, 1 + LG], BF16, tag="S", name="S")
                nc.scalar.copy(S[:, 0:1], Mv0T[:, m0 : m0 + 1])
                nc.vector.tensor_tensor_scan(
                    S[:, 1 : 1 + LG], alpha[:], beta[:],
                    Mv0T[:, m0 : m0 + 1], op0=AL.mult, op1=AL.add
                )
                # wm = S_shift * w: contiguous ins, strided slot in wm_pair
                wm3 = wm_pair[:].rearrange("p (m t) -> p m t", m=MH)[
                    :, :, slot * CH : slot * CH + CH]
                nc.vector.tensor_tensor(
                    wm3, S[:, 0:LG].rearrange("p (m t) -> p m t", m=MH),
                    w_bc3, op=AL.mult)

            pred = ph1.tile([1, BT], FP32, tag="pred")
            pair_state = {}

            def pair_h(p, h):
                b0 = 2 * p
                if h == 0:
                    pair_state[p] = (
                        ppF.tile([D, PG], FP32, tag="fps", name=f"f_ps{p}"),
                        wmp_p.tile([128, MH * PG], BF16, tag="wmp", name=f"wmp{p}"),
                    )
                f_ps, wm_pair = pair_state[p]
                wmp3 = wm_pair[:].rearrange("p (m t) -> p m t", m=MH)
                chains(b0, h, wm_pair, 0)
                chains(b0 + 1, h, wm_pair, 1)
                for m in range(MH):
                    nc.tensor.matmul(
                        f_ps[:], lhsT=fW1Tb[:], rhs=wmp3[:, m, :],
                        start=(h == 0 and m == 0), stop=False)
                if h == 1:
                    # + fW2 @ k per student, close the accumulation
                    nc.tensor.matmul(f_ps[:, 0:T], lhsT=fW2Tb[:],
                                     rhs=k_Tb[:, b0 * T : (b0 + 1) * T],
                                     start=False, stop=False)
                    nc.tensor.matmul(f_ps[:, CH : CH + T], lhsT=fW2Tb[:],
                                     rhs=k_Tb[:, (b0 + 1) * T : (b0 + 2) * T],
                                     start=False, stop=True)
                    nc.scalar.activation(f_sb[:, b0 * T : (b0 + 1) * T],
                                         f_ps[:, 0:T],
                                         AF.Tanh, bias=f_b[:], scale=1.0)
                    nc.scalar.activation(f_sb[:, (b0 + 1) * T : (b0 + 2) * T],
                                         f_ps[:, CH : CH + T],
                                         AF.Tanh, bias=f_b[:], scale=1.0)


            # interleaved schedule: chunk/tail units slotted between pair
            # halves so tensor-side phase-1 fills stalls inside each pair
            schedule = [
                ("c", 0), ("c", 1), ("c", 2), ("c", 3), ("t", 0),
                ("P", (0, 0)), ("c", 4), ("c", 5), ("P", (0, 1)),
                ("c", 6), ("t", 1),
                ("P", (1, 0)), ("c", 7), ("c", 8), ("P", (1, 1)),
                ("c", 9), ("t", 2),
                ("P", (2, 0)), ("c", 10), ("c", 11), ("P", (2, 1)),
                ("c", 12), ("t", 3),
                ("P", (3, 0)), ("P", (3, 1)),
            ]
            for kind, arg in schedule:
                if kind == "c":
                    phase1_chunk(arg)
                elif kind == "t":
                    phase1_tail(arg)
                elif kind == "w":
                    pw_chunk(arg)
                else:
                    pair_h(*arg)

            # ---- pred = sigmoid(sum_d f*pw + pb) ----
            nc.vector.tensor_mul(f_sb[:], f_sb[:], pw_T[:, :BT])
            for c in range(0, BT, 400):
                ppd = pp.tile([128, 1024], FP32, tag="mm", name=f"ppd{c}")
                nc.tensor.matmul(ppd[0:1, 0:400], lhsT=ones[:],
                                 rhs=f_sb[:, c : c + 400])
                nc.scalar.copy(pred[:, c : c + 400], ppd[0:1, 0:400])
            nc.vector.tensor_add(pred[:], pred[:], pb_sel[:])
            nc.scalar.activation(pred[:], pred[:], AF.Sigmoid)
            nc.sync.dma_start(out_d.ap().rearrange("b t -> (b t)").unsqueeze(0), pred[:])

    nc.compile()
    return nc


def _prep_inputs(skill, answer, k_emb, v_emb, Mk, Mv0, f_W, f_b, p_W, p_b,
                 e_W, e_b, a_W, a_b):
    skill = np.asarray(skill)
    answer = np.asarray(answer)
    answer_x = np.where(answer == 2, 1, answer)
    x = (skill + NUM_C * answer_x).astype(np.int64)
    nxt = np.concatenate([skill[:, 1:], np.zeros((B, 1), skill.dtype)], axis=1)
    pidx_full = np.minimum(nxt, NUM_C - 1).astype(np.int64)

    def idx_tiles(a):
        flat = np.zeros(BTP, np.int32)
        flat[:BT] = a.reshape(-1).astype(np.int32)
        return np.ascontiguousarray(flat.reshape(NCHUNK, 128).T)

    bf = ml_dtypes.bfloat16
    common = {
        "k_emb": np.ascontiguousarray(k_emb, np.float32),
        "v_emb": np.ascontiguousarray(v_emb, np.float32),
        "p_W": np.ascontiguousarray(p_W, np.float32),
        "MkT": np.ascontiguousarray(Mk.T, np.float32),
        "Mv0T": np.ascontiguousarray(Mv0.T, np.float32),
        "eWT": np.ascontiguousarray(e_W.T, np.float32),
        "aWT": np.ascontiguousarray(a_W.T, np.float32),
        "fW1Tb": np.ascontiguousarray(f_W[:, :D].T).astype(bf),
        "fW2Tb": np.ascontiguousarray(f_W[:, D:].T).astype(bf),
        "e_b": np.ascontiguousarray(e_b.reshape(D, 1), np.float32),
        "a_b": np.ascontiguousarray(a_b.reshape(D, 1), np.float32),
        "f_b": np.ascontiguousarray(f_b.reshape(D, 1), np.float32),
        "ident": np.eye(D, dtype=np.float32),
        "identb": np.eye(D, dtype=np.float32).astype(bf),
        "ones": np.ones((D, 1)).astype(bf),
        "ones_row": np.ones((1, D)).astype(ml_dtypes.bfloat16),
    }
    in_maps = []
    for c in range(N_CORES):
        sl = slice(c * BC, (c + 1) * BC)
        m = dict(common)
        m["kidx"] = idx_tiles(skill[sl])
        m["vidx"] = idx_tiles(x[sl])
        m["pidx"] = idx_tiles(pidx_full[sl])
        m["pb_sel"] = np.ascontiguousarray(
            np.asarray(p_b, np.float32)[pidx_full[sl]].reshape(1, BT))
        in_maps.append(m)
    return in_maps


def _install_ntff_hook_shim():
    """Provide antenv.axon_hooks with a ctypes NTFF profile hook when the
    container's antenv package lacks it (needed only for trace=True)."""
    import types
    import ctypes
    import contextlib

    try:
        from antenv.axon_hooks import get_axon_ntff_profile_hook  # noqa: F401
        return
    except ImportError:
        pass

    so_path = "/opt/axon/libaxon_pjrt.so"
    hook = None
    if os.path.exists(so_path):
        lib = ctypes.CDLL(so_path)
        if hasattr(lib, "axon_start_nrt_profile"):
            lib.axon_start_nrt_profile.argtypes = [
                ctypes.POINTER(ctypes.c_int64), ctypes.c_size_t]
            lib.axon_start_nrt_profile.restype = ctypes.c_int64
            lib.axon_stop_nrt_profile.argtypes = [ctypes.c_char_p]
            lib.axon_stop_nrt_profile.restype = ctypes.c_int64

            @contextlib.contextmanager
            def _hook(output_dir, device_ids):
                import jax
                jax.devices()
                if device_ids:
                    ids = (ctypes.c_int64 * len(device_ids))(*device_ids)
                    rc = lib.axon_start_nrt_profile(ids, len(device_ids))
                else:
                    rc = lib.axon_start_nrt_profile(None, 0)
                if rc != 0:
                    raise RuntimeError(f"axon_start_nrt_profile rc={rc}")
                try:
                    yield
                finally:
                    n = lib.axon_stop_nrt_profile(str(output_dir).encode())
                    print(f"profile: {n} file(s) written to {output_dir}",
                          file=sys.stderr)

            hook = _hook

    mod = types.ModuleType("antenv.axon_hooks")
    mod._hook = hook
    mod.get_axon_ntff_profile_hook = lambda: mod._hook
    mod.set_axon_ntff_profile_hook = lambda h: setattr(mod, "_hook", h)
    import antenv
    antenv.axon_hooks = mod
    sys.modules["antenv.axon_hooks"] = mod




def kernel(**inputs):
    if "nc" not in _COMPILED:
        _COMPILED["nc"] = _build_nc()
    nc = _COMPILED["nc"]
    in_maps = _prep_inputs(**inputs)
    trace = bool(int(os.environ.get("KERNEL_TRACE", "0")))
    if trace:
        _install_ntff_hook_shim()
    res = bass_utils.run_bass_kernel_spmd(
        nc, in_maps, core_ids=list(range(N_CORES)), trace=trace
    )
    _COMPILED["last_result"] = res
    out = np.concatenate([res.results[c]["out"][:, : T - 1] for c in range(N_CORES)], axis=0)
    return out.astype(np.float32)

